# revision 35
# baseline (speedup 1.0000x reference)
import sys

sys.path.insert(0, "/opt/trn_rl_repo")

import numpy as np

import concourse.bacc as bacc
import concourse.mybir as mybir
import concourse.tile as tile
from concourse.bass_utils import run_bass_kernel_spmd
from concourse.masks import make_identity

# Problem constants (nn_AMMConv2d: 3x3 conv via product quantization, STE forward)
NC, K, SUB = 16, 16, 72
CIN, COUT = 128, 256
H = W = 56
B = 8
PW = W + 2             # padded width 58
NT = 128               # positions per tile (flattened padded coords)
P0 = PW + 1            # first valid flat position (row 1, col 1) = 59
PLAST = W * PW + W     # last valid flat position = 3304
NTILES = (PLAST - P0 + 1 + NT - 1) // NT   # 26
XPL = 3456             # 1 guard col + 58*58 padded image + tail guard
CK = NC * K            # 256
# tap offsets in flattened padded coords (kh-1)*PW + (kw-1)
TAPS = [(kh - 1) * PW + (kw - 1) for kh in range(3) for kw in range(3)]
# xp DMA chunks in columns (first small for fast pipeline start)
XCHUNKS = [640, 512, 768, 896, 640]
# output tile groups (start, size); tail groups shrink to drain faster
_OG = [(0, 4), (4, 4), (8, 4), (12, 4), (16, 4), (20, 2), (22, 2), (24, 2)]
OGROUPS = {}
for _s, _n in _OG:
    for _t in range(_s, _s + _n):
        OGROUPS[_t] = (_s, _n)
OGRP = 4
NWARM = 36             # PE warmup transposes (p-state ramp while DMAs land)
LAST_W = 64            # width of the final (mostly-junk) tile

F32 = mybir.dt.float32
F32R = mybir.dt.float32r
F16 = mybir.dt.float16


def build():
    nc = bacc.Bacc("TRN2", target_bir_lowering=False, debug=False)

    xp_ext = nc.declare_dram_parameter("xp", [CIN, XPL], F32R, isOutput=False)
    cmm_ext = nc.declare_dram_parameter("cmm", [CIN, 9 * CK], F32R, isOutput=False)
    c2g_ext = nc.declare_dram_parameter("c2g", [CIN, CK], F32, isOutput=False)
    lut_ext = nc.declare_dram_parameter("lut", [CIN, 2 * COUT], F16, isOutput=False)
    idn_ext = nc.declare_dram_parameter("idn", [NT, NT], F16, isOutput=False)
    c2row_ext = nc.declare_dram_parameter(
        "c2row", [1, CK + NT], F32R, isOutput=False
    )
    # partition-major output: out[p, t, o]; host untangles valid rows
    out_ext = nc.declare_dram_parameter(
        "out", [CIN, NTILES * COUT], F16, isOutput=True
    )
    out3 = out_ext[:].rearrange("p (t o) -> p t o", t=NTILES)

    with tile.TileContext(nc) as tc:
        with (
            tc.tile_pool(name="const", bufs=1) as const_pool,
            tc.tile_pool(name="work", bufs=4) as work,
            tc.tile_pool(name="obuf", bufs=3) as obuf,
            tc.tile_pool(name="spsum", bufs=4, space="PSUM") as spsum,
            tc.tile_pool(name="tpsum", bufs=2, space="PSUM") as tpsum,
            tc.tile_pool(name="opsum", bufs=2, space="PSUM") as opsum,
        ):
            xp = const_pool.tile([CIN, XPL], F32R)
            cmm = const_pool.tile([CIN, 9, CK], F32R)
            c2g = const_pool.tile([CIN, CK], F32)
            lut = const_pool.tile([CIN, 2, COUT], F16)

            # identity first so PE warmup can start immediately
            ident = const_pool.tile([NT, NT], F16)
            c2row = const_pool.tile([1, CK + NT], F32R)

            cmm2 = cmm[:].rearrange("p a b -> p (a b)")
            # sync queue: identity, first xp chunk, taps 6-8, then the xp
            # tail chained via 1-col overlap so it yields bandwidth to taps
            nc.sync.dma_start(ident[:], idn_ext[:])
            nc.sync.dma_start(xp[:, 0 : XCHUNKS[0]], xp_ext[:, 0 : XCHUNKS[0]])
            nc.sync.dma_start(
                cmm2[:, 6 * CK : 9 * CK], cmm_ext[:, 6 * CK : 9 * CK]
            )
            c0 = XCHUNKS[0]
            for ncols in XCHUNKS[1:]:
                c1 = min(c0 + ncols, XPL)
                nc.sync.dma_start(xp[:, c0 - 1 : c1], xp_ext[:, c0 - 1 : c1])
                c0 = c1
            # scalar queue: tap 0 (gates the first matmul), taps 1-2, tap 5
            nc.scalar.dma_start(cmm2[:, 0:CK], cmm_ext[:, 0:CK])
            nc.scalar.dma_start(cmm2[:, CK : 3 * CK], cmm_ext[:, CK : 3 * CK])
            nc.scalar.dma_start(
                cmm2[:, 5 * CK : 6 * CK], cmm_ext[:, 5 * CK : 6 * CK]
            )
            nc.scalar.dma_start(c2row[:], c2row_ext[:])
            # gpsimd queue: taps 3-4, c2g, tap 5, lut (in order of first use)
            nc.gpsimd.dma_start(
                cmm2[:, 3 * CK : 5 * CK], cmm_ext[:, 3 * CK : 5 * CK]
            )
            nc.gpsimd.dma_start(c2g[:], c2g_ext[:])
            nc.gpsimd.dma_start(
                lut[:].rearrange("p a b -> p (a b)"), lut_ext[:]
            )

            # PE warmup: the tensor-engine clock ramps only after ~6us of
            # continuous activity, so start dummy transposes right after the
            # preamble (fed by a memset scratch, no DMA dependency) and
            # bridge until the first input data lands
            wsrc = const_pool.tile([NT, NT], F16)
            nc.gpsimd.memset(wsrc[:], 0)
            warm_ps = tpsum.tile([CIN, 2 * 2 * NT], F16, tag="mt", name="warm")
            for _ in range(NWARM):
                nc.tensor.transpose(warm_ps[:, :NT], wsrc[:], wsrc[:])

            def tw(t):
                return LAST_W if t == NTILES - 1 else NT

            ORDER = (0, 3, 4, 1, 2, 6, 7, 8, 5)  # tap DMA arrival order
            NHEAD = 4  # tiles processed tap-major while taps stream in

            # ---------- main loop: 4-deep software pipeline ----------
            def emit_scores(t):
                base = 1 + P0 + t * NT  # guard col + flat window start
                n = tw(t)
                fold = t >= NTILES - 4
                s_ps = spsum.tile([NT, CK], F32, tag="scores", name="s_ps")
                for i, kk in enumerate(ORDER):
                    off = TAPS[kk]
                    nc.tensor.matmul(
                        s_ps[:n, :],
                        xp[:, base + off : base + off + n],
                        cmm[:, kk, :],
                        start=(i == 0),
                        stop=(i == 8 and not fold),
                    )
                if fold:
                    # fold the -c2/2 bias into PSUM (tensor idles at the
                    # tail; removes the DVE add from the drain window)
                    nc.tensor.matmul(
                        s_ps[:n, :],
                        c2row[:, CK : CK + n],
                        c2row[:, :CK],
                        start=False,
                        stop=True,
                    )
                return s_ps

            def emit_scores_head():
                # arrival-group wavefront over the first NHEAD tiles: each
                # DMA group's matmuls for all head tiles fire as the group
                # lands, so head tiles complete right after the last tap
                # arrives instead of serializing behind it; tile-major
                # within a group keeps PSUM-bank switches low
                groups = ((0,), (3, 4), (1, 2), (6, 7, 8), (5,))
                tiles = [
                    spsum.tile([NT, CK], F32, tag="scores", name="s_ps")
                    for _ in range(NHEAD)
                ]
                ng = 0
                for gi, grp in enumerate(groups):
                    for t in range(NHEAD):
                        base = 1 + P0 + t * NT
                        for pos, kk in enumerate(grp):
                            i = ng + pos
                            off = TAPS[kk]
                            nc.tensor.matmul(
                                tiles[t][:],
                                xp[:, base + off : base + off + NT],
                                cmm[:, kk, :],
                                start=(i == 0),
                                stop=(i == 8),
                            )
                    ng += len(grp)
                return tiles

            def emit_epi_a(t, s_ps):
                # g = xc - c2/2; row-max per codebook; one-hot mask (DVE)
                n = tw(t)
                if t >= NTILES - 4:
                    # bias already folded into PSUM by the 10th matmul
                    g3 = s_ps[:n, :].rearrange("p (c k) -> p c k", c=NC)
                else:
                    g = work.tile([NT, CK], F32, tag="g", name="g")
                    nc.vector.tensor_tensor(
                        g[:n, :], s_ps[:n, :], c2g[:n, :], mybir.AluOpType.add
                    )
                    g3 = g[:n, :].rearrange("p (c k) -> p c k", c=NC)
                gmax = work.tile([NT, NC], F32, tag="gmax", name="gmax")
                nc.vector.tensor_reduce(
                    gmax[:n, :], g3, axis=mybir.AxisListType.X,
                    op=mybir.AluOpType.max
                )
                mask = work.tile([NT, CK], F16, tag="mask", name="mask")
                nc.vector.tensor_tensor(
                    mask[:n, :].rearrange("p (c k) -> p c k", c=NC),
                    g3,
                    gmax[:n, :].unsqueeze(2).broadcast_to([n, NC, K]),
                    mybir.AluOpType.is_equal,
                )
                return mask

            def emit_epi_b1(batch):
                # batch: list of (t, mask) pairs (1 or 2 tiles)
                nb = len(batch)
                mt_ps = tpsum.tile([CIN, 2 * 2 * NT], F16, tag="mt", name="mt_ps")
                for i, (t, mask) in enumerate(batch):
                    n = tw(t)
                    for j in range(2):
                        nc.tensor.transpose(
                            mt_ps[:, (2 * i + j) * NT : (2 * i + j) * NT + n],
                            mask[:n, j * CIN : (j + 1) * CIN],
                            ident[:n, :n],
                        )

                mT = work.tile([CIN, 2 * 2 * NT], F16, tag="mT", name="mT")
                nc.scalar.activation(
                    mT[:, : 2 * nb * NT],
                    mt_ps[:, : 2 * nb * NT],
                    mybir.ActivationFunctionType.Copy,
                )
                return mT

            def emit_epi_b2(batch, mT, o_sb):
                nb = len(batch)
                o_ps = opsum.tile([NT, 2 * COUT], F32, tag="out", name="o_ps")
                for i, (t, _) in enumerate(batch):
                    n = tw(t)
                    for j in range(2):
                        nc.tensor.matmul(
                            o_ps[:n, i * COUT : (i + 1) * COUT],
                            mT[:, (2 * i + j) * NT : (2 * i + j) * NT + n],
                            lut[:, j, :],
                            start=(j == 0),
                            stop=(j == 1),
                        )

                tA = batch[0][0]
                t0, gsz = OGROUPS[tA]
                slot = tA - t0
                nc.scalar.activation(
                    o_sb[:, slot * COUT : (slot + nb) * COUT],
                    o_ps[:, : nb * COUT],
                    mybir.ActivationFunctionType.Copy,
                )
                tlast = batch[-1][0]
                t0, gsz = OGROUPS[tlast]
                if tlast - t0 == gsz - 1:
                    nc.sync.dma_start(
                        out3[:, t0 : t0 + gsz, :],
                        o_sb[:, : gsz * COUT].rearrange(
                            "p (a b) -> p a b", a=gsz
                        ),
                    )

            def get_osb(t):
                if OGROUPS[t][0] == t:
                    get_osb.cur = obuf.tile(
                        [NT, OGRP * COUT], F16, tag="osb", name="o_sb"
                    )
                return get_osb.cur

            stage_a = []  # (t, s_ps)
            stage_b = []  # (t, mask) — paired into batches of 2
            stage_c = []  # (batch, mT)
            def pump(drain=False):
                if len(stage_a) > (0 if drain else 1):
                    ta, s_ps = stage_a.pop(0)
                    stage_b.append((ta, emit_epi_a(ta, s_ps)))
                take = 0
                if len(stage_b) >= 2:
                    take = 2
                elif stage_b and drain:
                    take = 1
                if take:
                    batch = stage_b[:take]
                    del stage_b[:take]
                    stage_c.append((batch, emit_epi_b1(batch)))
                if len(stage_c) > (0 if drain else 1):
                    batch, mT = stage_c.pop(0)
                    emit_epi_b2(batch, mT, get_osb(batch[0][0]))
            for t, s in enumerate(emit_scores_head()):
                stage_a.append((t, s))
            for t in range(NHEAD, NTILES):
                stage_a.append((t, emit_scores(t)))
                pump()
            while stage_a or stage_b or stage_c:
                pump(drain=True)

    nc.compile()
    return nc


def prep_consts(centroids, weight, bias):
    """Host-side constant packing (exact f32/f16; no device prologue math)."""
    centroids = np.asarray(centroids, dtype=np.float32)
    weight = np.asarray(weight, dtype=np.float32)
    bias = np.asarray(bias, dtype=np.float32)

    # cmm[8c+a, kk*CK + c*K + k] = centroids[c, k, a*9 + kk]
    cents_mm = np.zeros((9, CIN, CK), dtype=np.float32)
    cs = centroids.reshape(NC, K, 8, 9)  # s = a*9 + kk
    for c in range(NC):
        for a in range(8):
            cents_mm[:, 8 * c + a, c * K : (c + 1) * K] = cs[c, :, a, :].T
    cmm = np.ascontiguousarray(cents_mm.transpose(1, 0, 2).reshape(CIN, 9 * CK))

    c2 = (centroids * centroids).sum(-1).reshape(CK)  # [NC*K]
    c2g = np.ascontiguousarray(
        np.broadcast_to((-0.5 * c2)[None, :], (CIN, CK))
    ).astype(np.float32)
    c2row = np.zeros((1, CK + NT), dtype=np.float32)
    c2row[0, :CK] = -0.5 * c2
    c2row[0, CK:] = 1.0

    # lut[c*K+k, o] = (centroids[c] @ weight[c])[k, o] + bias[o]/NC
    lut_full = np.einsum("cks,cso->cko", centroids, weight).reshape(CK, COUT)
    lut_full = lut_full + bias[None, :] / NC
    lut2 = np.concatenate([lut_full[:CIN], lut_full[CIN:]], axis=1)  # [128, 512]
    lut2 = np.ascontiguousarray(lut2).astype(np.float16)
    return cmm, c2g, lut2, c2row


def prep_x(xi):
    xp = np.zeros((CIN, XPL), dtype=np.float32)
    xp[:, 1 : 1 + PW * PW] = np.pad(xi, ((0, 0), (1, 1), (1, 1))).reshape(
        CIN, PW * PW
    )
    return xp


def prep_in_maps(x, centroids, weight, bias):
    x = np.asarray(x, dtype=np.float32)
    cmm, c2g, lut2, c2row = prep_consts(centroids, weight, bias)
    idn = np.eye(NT, dtype=np.float16)
    return [
        {
            "xp": prep_x(x[i]),
            "cmm": cmm,
            "c2g": c2g,
            "lut": lut2,
            "idn": idn,
            "c2row": c2row,
        }
        for i in range(B)
    ]


# valid-position selector over the 26*128 flat slots
_PFLAT = np.arange(P0, P0 + NTILES * NT)
_PSEL = (_PFLAT <= PLAST) & (_PFLAT % PW >= 1) & (_PFLAT % PW <= W)


def unpack_out(raw):
    """raw [CIN, NTILES*COUT] f16 -> [COUT, H, W] f32 for one image."""
    arr = np.asarray(raw, dtype=np.float32).reshape(CIN, NTILES, COUT)
    a = arr.transpose(1, 0, 2).reshape(NTILES * NT, COUT)  # flat slot-major
    return a[_PSEL].reshape(H, W, COUT).transpose(2, 0, 1)


_NC_CACHE = []


def kernel(x, centroids, weight, inverse_temperature_logit, bias):
    if not _NC_CACHE:
        _NC_CACHE.append(build())
    nc = _NC_CACHE[0]

    in_maps = prep_in_maps(x, centroids, weight, bias)
    res = run_bass_kernel_spmd(nc, in_maps, core_ids=list(range(B)))
    out = np.stack([unpack_out(res.results[i]["out"]) for i in range(B)])
    return np.ascontiguousarray(out.astype(np.float32))


# revision 36
# speedup vs baseline: 1.0025x; 1.0025x over previous
import sys

sys.path.insert(0, "/opt/trn_rl_repo")

import numpy as np

import concourse.bacc as bacc
import concourse.mybir as mybir
import concourse.tile as tile
from concourse.bass_utils import run_bass_kernel_spmd
from concourse.masks import make_identity

# Problem constants (nn_AMMConv2d: 3x3 conv via product quantization, STE forward)
NC, K, SUB = 16, 16, 72
CIN, COUT = 128, 256
H = W = 56
B = 8
PW = W + 2             # padded width 58
NT = 128               # positions per tile (flattened padded coords)
P0 = PW + 1            # first valid flat position (row 1, col 1) = 59
PLAST = W * PW + W     # last valid flat position = 3304
NTILES = (PLAST - P0 + 1 + NT - 1) // NT   # 26
XPL = 3456             # 1 guard col + 58*58 padded image + tail guard
CK = NC * K            # 256
# tap offsets in flattened padded coords (kh-1)*PW + (kw-1)
TAPS = [(kh - 1) * PW + (kw - 1) for kh in range(3) for kw in range(3)]
# xp DMA chunks in columns (first small for fast pipeline start)
XCHUNKS = [640, 512, 768, 896, 640]
# output tile groups (start, size); tail groups shrink to drain faster
_OG = [(0, 4), (4, 4), (8, 4), (12, 4), (16, 4), (20, 2), (22, 2), (24, 2)]
OGROUPS = {}
for _s, _n in _OG:
    for _t in range(_s, _s + _n):
        OGROUPS[_t] = (_s, _n)
OGRP = 4
NWARM = 36             # PE warmup transposes (p-state ramp while DMAs land)
LAST_W = 64            # width of the final (mostly-junk) tile

F32 = mybir.dt.float32
F32R = mybir.dt.float32r
F16 = mybir.dt.float16


def build():
    nc = bacc.Bacc("TRN2", target_bir_lowering=False, debug=False)

    xp_ext = nc.declare_dram_parameter("xp", [CIN, XPL], F32R, isOutput=False)
    cmm_ext = nc.declare_dram_parameter("cmm", [CIN, 9 * CK], F32R, isOutput=False)
    c2g_ext = nc.declare_dram_parameter("c2g", [CIN, CK], F32, isOutput=False)
    lut_ext = nc.declare_dram_parameter("lut", [CIN, 2 * COUT], F16, isOutput=False)
    idn_ext = nc.declare_dram_parameter("idn", [NT, NT], F16, isOutput=False)
    c2row_ext = nc.declare_dram_parameter(
        "c2row", [1, 2 * CK + NT], F32R, isOutput=False
    )
    # partition-major output: out[p, t, o]; host untangles valid rows
    out_ext = nc.declare_dram_parameter(
        "out", [CIN, NTILES * COUT], F16, isOutput=True
    )
    out3 = out_ext[:].rearrange("p (t o) -> p t o", t=NTILES)

    with tile.TileContext(nc) as tc:
        with (
            tc.tile_pool(name="const", bufs=1) as const_pool,
            tc.tile_pool(name="work", bufs=4) as work,
            tc.tile_pool(name="obuf", bufs=3) as obuf,
            tc.tile_pool(name="spsum", bufs=4, space="PSUM") as spsum,
            tc.tile_pool(name="tpsum", bufs=2, space="PSUM") as tpsum,
            tc.tile_pool(name="opsum", bufs=2, space="PSUM") as opsum,
        ):
            xp = const_pool.tile([CIN, XPL], F32R)
            cmm = const_pool.tile([CIN, 9, CK], F32R)
            c2g = const_pool.tile([CIN, CK], F32)
            lut = const_pool.tile([CIN, 2, COUT], F16)

            # identity first so PE warmup can start immediately
            ident = const_pool.tile([NT, NT], F16)
            c2row = const_pool.tile([1, 2 * CK + NT], F32R)

            cmm2 = cmm[:].rearrange("p a b -> p (a b)")
            # sync queue: identity, first xp chunk, taps 6-8, then the xp
            # tail chained via 1-col overlap so it yields bandwidth to taps
            nc.sync.dma_start(ident[:], idn_ext[:])
            nc.sync.dma_start(xp[:, 0 : XCHUNKS[0]], xp_ext[:, 0 : XCHUNKS[0]])
            nc.sync.dma_start(
                cmm2[:, 6 * CK : 9 * CK], cmm_ext[:, 6 * CK : 9 * CK]
            )
            c0 = XCHUNKS[0]
            for ncols in XCHUNKS[1:]:
                c1 = min(c0 + ncols, XPL)
                nc.sync.dma_start(xp[:, c0 - 1 : c1], xp_ext[:, c0 - 1 : c1])
                c0 = c1
            # scalar queue: tap 0 (gates the first matmul), taps 1-2, tap 5
            nc.scalar.dma_start(cmm2[:, 0:CK], cmm_ext[:, 0:CK])
            nc.scalar.dma_start(cmm2[:, CK : 3 * CK], cmm_ext[:, CK : 3 * CK])
            nc.scalar.dma_start(
                cmm2[:, 5 * CK : 6 * CK], cmm_ext[:, 5 * CK : 6 * CK]
            )
            nc.scalar.dma_start(c2row[:], c2row_ext[:])
            # gpsimd queue: taps 3-4, c2g, tap 5, lut (in order of first use)
            nc.gpsimd.dma_start(
                cmm2[:, 3 * CK : 5 * CK], cmm_ext[:, 3 * CK : 5 * CK]
            )
            nc.gpsimd.dma_start(c2g[:], c2g_ext[:])
            nc.gpsimd.dma_start(
                lut[:].rearrange("p a b -> p (a b)"), lut_ext[:]
            )

            # PE warmup: the tensor-engine clock ramps only after ~6us of
            # continuous activity, so start dummy transposes right after the
            # preamble (fed by a memset scratch, no DMA dependency) and
            # bridge until the first input data lands
            wsrc = const_pool.tile([NT, NT], F16)
            nc.gpsimd.memset(wsrc[:], 0)
            warm_ps = tpsum.tile([CIN, 2 * 2 * NT], F16, tag="mt", name="warm")
            for _ in range(NWARM):
                nc.tensor.transpose(warm_ps[:, :NT], wsrc[:], wsrc[:])

            def tw(t):
                return LAST_W if t == NTILES - 1 else NT

            ORDER = (0, 3, 4, 1, 2, 6, 7, 8, 5)  # tap DMA arrival order
            NHEAD = 4  # tiles processed tap-major while taps stream in

            # ---------- main loop: 4-deep software pipeline ----------
            def emit_scores(t):
                base = 1 + P0 + t * NT  # guard col + flat window start
                n = tw(t)
                fold = t >= NTILES - 4
                s_ps = spsum.tile([NT, CK], F32, tag="scores", name="s_ps")
                for i, kk in enumerate(ORDER):
                    off = TAPS[kk]
                    nc.tensor.matmul(
                        s_ps[:n, :],
                        xp[:, base + off : base + off + n],
                        cmm[:, kk, :],
                        start=(i == 0),
                        stop=(i == 8 and not fold),
                    )
                if fold:
                    # fold the -c2/2 bias into PSUM (tensor idles at the
                    # tail; removes the DVE add from the drain window);
                    # two bf16-split pieces keep the tf32 fold exact
                    for h in range(2):
                        nc.tensor.matmul(
                            s_ps[:n, :],
                            c2row[:, 2 * CK : 2 * CK + n],
                            c2row[:, h * CK : (h + 1) * CK],
                            start=False,
                            stop=(h == 1),
                        )
                return s_ps

            def emit_scores_head():
                # arrival-group wavefront over the first NHEAD tiles: each
                # DMA group's matmuls for all head tiles fire as the group
                # lands, so head tiles complete right after the last tap
                # arrives instead of serializing behind it; tile-major
                # within a group keeps PSUM-bank switches low
                groups = ((0,), (3, 4), (1, 2), (6, 7, 8), (5,))
                tiles = [
                    spsum.tile([NT, CK], F32, tag="scores", name="s_ps")
                    for _ in range(NHEAD)
                ]
                ng = 0
                for gi, grp in enumerate(groups):
                    for t in range(NHEAD):
                        base = 1 + P0 + t * NT
                        for pos, kk in enumerate(grp):
                            i = ng + pos
                            off = TAPS[kk]
                            nc.tensor.matmul(
                                tiles[t][:],
                                xp[:, base + off : base + off + NT],
                                cmm[:, kk, :],
                                start=(i == 0),
                                stop=(i == 8),
                            )
                    ng += len(grp)
                return tiles

            def emit_epi_a(t, s_ps):
                # g = xc - c2/2; row-max per codebook; one-hot mask (DVE)
                n = tw(t)
                if t >= NTILES - 4:
                    # bias already folded into PSUM by the 10th matmul
                    g3 = s_ps[:n, :].rearrange("p (c k) -> p c k", c=NC)
                else:
                    g = work.tile([NT, CK], F32, tag="g", name="g")
                    nc.vector.tensor_tensor(
                        g[:n, :], s_ps[:n, :], c2g[:n, :], mybir.AluOpType.add
                    )
                    g3 = g[:n, :].rearrange("p (c k) -> p c k", c=NC)
                gmax = work.tile([NT, NC], F32, tag="gmax", name="gmax")
                nc.vector.tensor_reduce(
                    gmax[:n, :], g3, axis=mybir.AxisListType.X,
                    op=mybir.AluOpType.max
                )
                mask = work.tile([NT, CK], F16, tag="mask", name="mask")
                nc.vector.tensor_tensor(
                    mask[:n, :].rearrange("p (c k) -> p c k", c=NC),
                    g3,
                    gmax[:n, :].unsqueeze(2).broadcast_to([n, NC, K]),
                    mybir.AluOpType.is_equal,
                )
                return mask

            def emit_epi_b1(batch):
                # batch: list of (t, mask) pairs (1 or 2 tiles)
                nb = len(batch)
                mt_ps = tpsum.tile([CIN, 2 * 2 * NT], F16, tag="mt", name="mt_ps")
                for i, (t, mask) in enumerate(batch):
                    n = tw(t)
                    for j in range(2):
                        nc.tensor.transpose(
                            mt_ps[:, (2 * i + j) * NT : (2 * i + j) * NT + n],
                            mask[:n, j * CIN : (j + 1) * CIN],
                            ident[:n, :n],
                        )

                mT = work.tile([CIN, 2 * 2 * NT], F16, tag="mT", name="mT")
                nc.scalar.activation(
                    mT[:, : 2 * nb * NT],
                    mt_ps[:, : 2 * nb * NT],
                    mybir.ActivationFunctionType.Copy,
                )
                return mT

            def emit_epi_b2(batch, mT, o_sb):
                nb = len(batch)
                o_ps = opsum.tile([NT, 2 * COUT], F32, tag="out", name="o_ps")
                for i, (t, _) in enumerate(batch):
                    n = tw(t)
                    for j in range(2):
                        nc.tensor.matmul(
                            o_ps[:n, i * COUT : (i + 1) * COUT],
                            mT[:, (2 * i + j) * NT : (2 * i + j) * NT + n],
                            lut[:, j, :],
                            start=(j == 0),
                            stop=(j == 1),
                        )

                tA = batch[0][0]
                t0, gsz = OGROUPS[tA]
                slot = tA - t0
                nc.scalar.activation(
                    o_sb[:, slot * COUT : (slot + nb) * COUT],
                    o_ps[:, : nb * COUT],
                    mybir.ActivationFunctionType.Copy,
                )
                tlast = batch[-1][0]
                t0, gsz = OGROUPS[tlast]
                if tlast - t0 == gsz - 1:
                    nc.sync.dma_start(
                        out3[:, t0 : t0 + gsz, :],
                        o_sb[:, : gsz * COUT].rearrange(
                            "p (a b) -> p a b", a=gsz
                        ),
                    )

            def get_osb(t):
                if OGROUPS[t][0] == t:
                    get_osb.cur = obuf.tile(
                        [NT, OGRP * COUT], F16, tag="osb", name="o_sb"
                    )
                return get_osb.cur

            stage_a = []  # (t, s_ps)
            stage_b = []  # (t, mask) — paired into batches of 2
            stage_c = []  # (batch, mT)
            def pump(drain=False):
                if len(stage_a) > (0 if drain else 1):
                    ta, s_ps = stage_a.pop(0)
                    stage_b.append((ta, emit_epi_a(ta, s_ps)))
                take = 0
                if len(stage_b) >= 2:
                    take = 2
                elif stage_b and drain:
                    take = 1
                if take:
                    batch = stage_b[:take]
                    del stage_b[:take]
                    stage_c.append((batch, emit_epi_b1(batch)))
                if len(stage_c) > (0 if drain else 1):
                    batch, mT = stage_c.pop(0)
                    emit_epi_b2(batch, mT, get_osb(batch[0][0]))
            for t, s in enumerate(emit_scores_head()):
                stage_a.append((t, s))
            for t in range(NHEAD, NTILES):
                stage_a.append((t, emit_scores(t)))
                pump()
            while stage_a or stage_b or stage_c:
                pump(drain=True)

    nc.compile()
    return nc


def prep_consts(centroids, weight, bias):
    """Host-side constant packing (exact f32/f16; no device prologue math)."""
    centroids = np.asarray(centroids, dtype=np.float32)
    weight = np.asarray(weight, dtype=np.float32)
    bias = np.asarray(bias, dtype=np.float32)

    # cmm[8c+a, kk*CK + c*K + k] = centroids[c, k, a*9 + kk]
    cents_mm = np.zeros((9, CIN, CK), dtype=np.float32)
    cs = centroids.reshape(NC, K, 8, 9)  # s = a*9 + kk
    for c in range(NC):
        for a in range(8):
            cents_mm[:, 8 * c + a, c * K : (c + 1) * K] = cs[c, :, a, :].T
    cmm = np.ascontiguousarray(cents_mm.transpose(1, 0, 2).reshape(CIN, 9 * CK))

    c2 = (centroids * centroids).sum(-1).reshape(CK)  # [NC*K]
    c2g = np.ascontiguousarray(
        np.broadcast_to((-0.5 * c2)[None, :], (CIN, CK))
    ).astype(np.float32)
    # -c2/2 split into bf16 hi + remainder lo so the f32r (tf32) matmul
    # fold is exact to ~2^-19; plus a ones row for the broadcast trick
    c2h = (-0.5 * c2).astype(np.float32)
    hi = c2h.astype(np.dtype("bfloat16") if hasattr(np, "bfloat16") else None)         if False else (
        np.frombuffer(
            (c2h.view(np.uint32) & np.uint32(0xFFFF0000)).tobytes(),
            dtype=np.float32,
        )
    )
    lo = c2h - hi
    c2row = np.zeros((1, 2 * CK + NT), dtype=np.float32)
    c2row[0, :CK] = hi
    c2row[0, CK : 2 * CK] = lo
    c2row[0, 2 * CK :] = 1.0

    # lut[c*K+k, o] = (centroids[c] @ weight[c])[k, o] + bias[o]/NC
    lut_full = np.einsum("cks,cso->cko", centroids, weight).reshape(CK, COUT)
    lut_full = lut_full + bias[None, :] / NC
    lut2 = np.concatenate([lut_full[:CIN], lut_full[CIN:]], axis=1)  # [128, 512]
    lut2 = np.ascontiguousarray(lut2).astype(np.float16)
    return cmm, c2g, lut2, c2row


def prep_x(xi):
    xp = np.zeros((CIN, XPL), dtype=np.float32)
    xp[:, 1 : 1 + PW * PW] = np.pad(xi, ((0, 0), (1, 1), (1, 1))).reshape(
        CIN, PW * PW
    )
    return xp


def prep_in_maps(x, centroids, weight, bias):
    x = np.asarray(x, dtype=np.float32)
    cmm, c2g, lut2, c2row = prep_consts(centroids, weight, bias)
    idn = np.eye(NT, dtype=np.float16)
    return [
        {
            "xp": prep_x(x[i]),
            "cmm": cmm,
            "c2g": c2g,
            "lut": lut2,
            "idn": idn,
            "c2row": c2row,
        }
        for i in range(B)
    ]


# valid-position selector over the 26*128 flat slots
_PFLAT = np.arange(P0, P0 + NTILES * NT)
_PSEL = (_PFLAT <= PLAST) & (_PFLAT % PW >= 1) & (_PFLAT % PW <= W)


def unpack_out(raw):
    """raw [CIN, NTILES*COUT] f16 -> [COUT, H, W] f32 for one image."""
    arr = np.asarray(raw, dtype=np.float32).reshape(CIN, NTILES, COUT)
    a = arr.transpose(1, 0, 2).reshape(NTILES * NT, COUT)  # flat slot-major
    return a[_PSEL].reshape(H, W, COUT).transpose(2, 0, 1)


_NC_CACHE = []


def kernel(x, centroids, weight, inverse_temperature_logit, bias):
    if not _NC_CACHE:
        _NC_CACHE.append(build())
    nc = _NC_CACHE[0]

    in_maps = prep_in_maps(x, centroids, weight, bias)
    res = run_bass_kernel_spmd(nc, in_maps, core_ids=list(range(B)))
    out = np.stack([unpack_out(res.results[i]["out"]) for i in range(B)])
    return np.ascontiguousarray(out.astype(np.float32))


# revision 37
# speedup vs baseline: 1.0479x; 1.0453x over previous
import sys

sys.path.insert(0, "/opt/trn_rl_repo")

import numpy as np

import concourse.bacc as bacc
import concourse.mybir as mybir
import concourse.tile as tile
from concourse.bass_utils import run_bass_kernel_spmd
from concourse.masks import make_identity

# Problem constants (nn_AMMConv2d: 3x3 conv via product quantization, STE forward)
NC, K, SUB = 16, 16, 72
CIN, COUT = 128, 256
H = W = 56
B = 8
PW = W + 2             # padded width 58
NT = 128               # positions per tile (flattened padded coords)
P0 = PW + 1            # first valid flat position (row 1, col 1) = 59
PLAST = W * PW + W     # last valid flat position = 3304
NTILES = (PLAST - P0 + 1 + NT - 1) // NT   # 26
XPL = 3456             # 1 guard col + 58*58 padded image + tail guard
CK = NC * K            # 256
# tap offsets in flattened padded coords (kh-1)*PW + (kw-1)
TAPS = [(kh - 1) * PW + (kw - 1) for kh in range(3) for kw in range(3)]
# xp DMA chunks in columns (first small for fast pipeline start)
XCHUNKS = [640, 512, 768, 896, 640]
# output tile groups (start, size); tail groups shrink to drain faster
_OG = [(0, 4), (4, 4), (8, 4), (12, 4), (16, 4), (20, 2), (22, 2), (24, 2)]
OGROUPS = {}
for _s, _n in _OG:
    for _t in range(_s, _s + _n):
        OGROUPS[_t] = (_s, _n)
OGRP = 4
NWARM = 36             # PE warmup transposes (p-state ramp while DMAs land)
LAST_W = 64            # width of the final (mostly-junk) tile

F32 = mybir.dt.float32
F32R = mybir.dt.float32r
F16 = mybir.dt.float16


def build():
    nc = bacc.Bacc("TRN2", target_bir_lowering=False, debug=False)

    xp_ext = nc.declare_dram_parameter("xp", [CIN, XPL], F32R, isOutput=False)
    cmm_ext = nc.declare_dram_parameter("cmm", [CIN, 9 * CK], F32R, isOutput=False)
    c2g_ext = nc.declare_dram_parameter("c2g", [CIN, CK], F32, isOutput=False)
    lut_ext = nc.declare_dram_parameter("lut", [CIN, 2 * COUT], F16, isOutput=False)
    idn_ext = nc.declare_dram_parameter("idn", [NT, NT], F16, isOutput=False)
    c2row_ext = nc.declare_dram_parameter(
        "c2row", [1, 2 * CK + NT], F32R, isOutput=False
    )
    # partition-major output: out[p, t, o]; host untangles valid rows
    out_ext = nc.declare_dram_parameter(
        "out", [CIN, NTILES * COUT], F16, isOutput=True
    )
    out3 = out_ext[:].rearrange("p (t o) -> p t o", t=NTILES)

    with tile.TileContext(nc) as tc:
        with (
            tc.tile_pool(name="const", bufs=1) as const_pool,
            tc.tile_pool(name="work", bufs=4) as work,
            tc.tile_pool(name="obuf", bufs=3) as obuf,
            tc.tile_pool(name="spsum", bufs=4, space="PSUM") as spsum,
            tc.tile_pool(name="tpsum", bufs=2, space="PSUM") as tpsum,
            tc.tile_pool(name="opsum", bufs=2, space="PSUM") as opsum,
        ):
            xp = const_pool.tile([CIN, XPL], F32R)
            cmm = const_pool.tile([CIN, 9, CK], F32R)
            c2g = const_pool.tile([CIN, CK], F32)
            lut = const_pool.tile([CIN, 2, COUT], F16)

            # identity first so PE warmup can start immediately
            ident = const_pool.tile([NT, NT], F16)
            c2row = const_pool.tile([1, 2 * CK + NT], F32R)

            cmm2 = cmm[:].rearrange("p a b -> p (a b)")
            # sync queue: identity, first xp chunk, taps 6-8, then the xp
            # tail chained via 1-col overlap so it yields bandwidth to taps
            nc.sync.dma_start(ident[:], idn_ext[:])
            nc.sync.dma_start(xp[:, 0 : XCHUNKS[0]], xp_ext[:, 0 : XCHUNKS[0]])
            nc.sync.dma_start(
                cmm2[:, 6 * CK : 9 * CK], cmm_ext[:, 6 * CK : 9 * CK]
            )
            c0 = XCHUNKS[0]
            for ncols in XCHUNKS[1:]:
                c1 = min(c0 + ncols, XPL)
                nc.sync.dma_start(xp[:, c0 - 1 : c1], xp_ext[:, c0 - 1 : c1])
                c0 = c1
            # scalar queue: tap 0 (gates the first matmul), taps 1-2, tap 5
            nc.scalar.dma_start(cmm2[:, 0:CK], cmm_ext[:, 0:CK])
            nc.scalar.dma_start(cmm2[:, CK : 3 * CK], cmm_ext[:, CK : 3 * CK])
            nc.scalar.dma_start(
                cmm2[:, 5 * CK : 6 * CK], cmm_ext[:, 5 * CK : 6 * CK]
            )
            nc.scalar.dma_start(c2row[:], c2row_ext[:])
            # gpsimd queue: taps 3-4, c2g, tap 5, lut (in order of first use)
            nc.gpsimd.dma_start(
                cmm2[:, 3 * CK : 5 * CK], cmm_ext[:, 3 * CK : 5 * CK]
            )
            nc.gpsimd.dma_start(c2g[:], c2g_ext[:])
            nc.gpsimd.dma_start(
                lut[:].rearrange("p a b -> p (a b)"), lut_ext[:]
            )

            # PE warmup: the tensor-engine clock ramps only after ~6us of
            # continuous activity, so start dummy transposes right after the
            # preamble (fed by a memset scratch, no DMA dependency) and
            # bridge until the first input data lands
            wsrc = const_pool.tile([NT, NT], F16)
            nc.gpsimd.memset(wsrc[:], 0)
            warm_ps = tpsum.tile([CIN, 2 * 2 * NT], F16, tag="mt", name="warm")
            for _ in range(NWARM):
                nc.tensor.transpose(warm_ps[:, :NT], wsrc[:], wsrc[:])

            def tw(t):
                return LAST_W if t == NTILES - 1 else NT

            ORDER = (0, 3, 4, 1, 2, 6, 7, 8, 5)  # tap DMA arrival order
            NHEAD = 4  # tiles processed tap-major while taps stream in

            # ---------- main loop: 4-deep software pipeline ----------
            def emit_scores(t):
                base = 1 + P0 + t * NT  # guard col + flat window start
                n = tw(t)
                fold = False
                s_ps = spsum.tile([NT, CK], F32, tag="scores", name="s_ps")
                for i, kk in enumerate(ORDER):
                    off = TAPS[kk]
                    nc.tensor.matmul(
                        s_ps[:n, :],
                        xp[:, base + off : base + off + n],
                        cmm[:, kk, :],
                        start=(i == 0),
                        stop=(i == 8 and not fold),
                    )
                if fold:
                    # fold the -c2/2 bias into PSUM (tensor idles at the
                    # tail; removes the DVE add from the drain window);
                    # two bf16-split pieces keep the tf32 fold exact
                    for h in range(2):
                        nc.tensor.matmul(
                            s_ps[:n, :],
                            c2row[:, 2 * CK : 2 * CK + n],
                            c2row[:, h * CK : (h + 1) * CK],
                            start=False,
                            stop=(h == 1),
                        )
                return s_ps

            def emit_scores_head():
                # arrival-group wavefront over the first NHEAD tiles: each
                # DMA group's matmuls for all head tiles fire as the group
                # lands, so head tiles complete right after the last tap
                # arrives instead of serializing behind it; tile-major
                # within a group keeps PSUM-bank switches low
                groups = ((0,), (3, 4), (1, 2), (6, 7, 8), (5,))
                tiles = [
                    spsum.tile([NT, CK], F32, tag="scores", name="s_ps")
                    for _ in range(NHEAD)
                ]
                ng = 0
                for gi, grp in enumerate(groups):
                    for t in range(NHEAD):
                        base = 1 + P0 + t * NT
                        for pos, kk in enumerate(grp):
                            i = ng + pos
                            off = TAPS[kk]
                            nc.tensor.matmul(
                                tiles[t][:],
                                xp[:, base + off : base + off + NT],
                                cmm[:, kk, :],
                                start=(i == 0),
                                stop=(i == 8),
                            )
                    ng += len(grp)
                return tiles

            def emit_epi_a(t, s_ps):
                # g = xc - c2/2; row-max per codebook; one-hot mask (DVE)
                n = tw(t)
                if False:
                    g3 = None
                else:
                    g = work.tile([NT, CK], F32, tag="g", name="g")
                    nc.vector.tensor_tensor(
                        g[:n, :], s_ps[:n, :], c2g[:n, :], mybir.AluOpType.add
                    )
                    g3 = g[:n, :].rearrange("p (c k) -> p c k", c=NC)
                gmax = work.tile([NT, NC], F32, tag="gmax", name="gmax")
                nc.vector.tensor_reduce(
                    gmax[:n, :], g3, axis=mybir.AxisListType.X,
                    op=mybir.AluOpType.max
                )
                mask = work.tile([NT, CK], F16, tag="mask", name="mask")
                nc.vector.tensor_tensor(
                    mask[:n, :].rearrange("p (c k) -> p c k", c=NC),
                    g3,
                    gmax[:n, :].unsqueeze(2).broadcast_to([n, NC, K]),
                    mybir.AluOpType.is_equal,
                )
                return mask

            def emit_epi_b1(batch):
                # batch: list of (t, mask) pairs (1 or 2 tiles)
                nb = len(batch)
                mt_ps = tpsum.tile([CIN, 2 * 2 * NT], F16, tag="mt", name="mt_ps")
                for i, (t, mask) in enumerate(batch):
                    n = tw(t)
                    for j in range(2):
                        nc.tensor.transpose(
                            mt_ps[:, (2 * i + j) * NT : (2 * i + j) * NT + n],
                            mask[:n, j * CIN : (j + 1) * CIN],
                            ident[:n, :n],
                        )

                mT = work.tile([CIN, 2 * 2 * NT], F16, tag="mT", name="mT")
                nc.scalar.activation(
                    mT[:, : 2 * nb * NT],
                    mt_ps[:, : 2 * nb * NT],
                    mybir.ActivationFunctionType.Copy,
                )
                return mT

            def emit_epi_b2(batch, mT, o_sb):
                nb = len(batch)
                o_ps = opsum.tile([NT, 2 * COUT], F32, tag="out", name="o_ps")
                for i, (t, _) in enumerate(batch):
                    n = tw(t)
                    for j in range(2):
                        nc.tensor.matmul(
                            o_ps[:n, i * COUT : (i + 1) * COUT],
                            mT[:, (2 * i + j) * NT : (2 * i + j) * NT + n],
                            lut[:, j, :],
                            start=(j == 0),
                            stop=(j == 1),
                        )

                tA = batch[0][0]
                t0, gsz = OGROUPS[tA]
                slot = tA - t0
                nc.scalar.activation(
                    o_sb[:, slot * COUT : (slot + nb) * COUT],
                    o_ps[:, : nb * COUT],
                    mybir.ActivationFunctionType.Copy,
                )
                tlast = batch[-1][0]
                t0, gsz = OGROUPS[tlast]
                if tlast - t0 == gsz - 1:
                    nc.sync.dma_start(
                        out3[:, t0 : t0 + gsz, :],
                        o_sb[:, : gsz * COUT].rearrange(
                            "p (a b) -> p a b", a=gsz
                        ),
                    )

            def get_osb(t):
                if OGROUPS[t][0] == t:
                    get_osb.cur = obuf.tile(
                        [NT, OGRP * COUT], F16, tag="osb", name="o_sb"
                    )
                return get_osb.cur

            stage_a = []  # (t, s_ps)
            stage_b = []  # (t, mask) — paired into batches of 2
            stage_c = []  # (batch, mT)
            def pump(drain=False):
                if len(stage_a) > (0 if drain else 1):
                    ta, s_ps = stage_a.pop(0)
                    stage_b.append((ta, emit_epi_a(ta, s_ps)))
                take = 0
                if len(stage_b) >= 2:
                    take = 2
                elif stage_b and drain:
                    take = 1
                if take:
                    batch = stage_b[:take]
                    del stage_b[:take]
                    stage_c.append((batch, emit_epi_b1(batch)))
                if len(stage_c) > (0 if drain else 1):
                    batch, mT = stage_c.pop(0)
                    emit_epi_b2(batch, mT, get_osb(batch[0][0]))
            for t, s in enumerate(emit_scores_head()):
                stage_a.append((t, s))
            for t in range(NHEAD, NTILES):
                stage_a.append((t, emit_scores(t)))
                pump()
            while stage_a or stage_b or stage_c:
                pump(drain=True)

    nc.compile()
    return nc


def prep_consts(centroids, weight, bias):
    """Host-side constant packing (exact f32/f16; no device prologue math)."""
    centroids = np.asarray(centroids, dtype=np.float32)
    weight = np.asarray(weight, dtype=np.float32)
    bias = np.asarray(bias, dtype=np.float32)

    # cmm[8c+a, kk*CK + c*K + k] = centroids[c, k, a*9 + kk]
    cents_mm = np.zeros((9, CIN, CK), dtype=np.float32)
    cs = centroids.reshape(NC, K, 8, 9)  # s = a*9 + kk
    for c in range(NC):
        for a in range(8):
            cents_mm[:, 8 * c + a, c * K : (c + 1) * K] = cs[c, :, a, :].T
    cmm = np.ascontiguousarray(cents_mm.transpose(1, 0, 2).reshape(CIN, 9 * CK))

    c2 = (centroids * centroids).sum(-1).reshape(CK)  # [NC*K]
    c2g = np.ascontiguousarray(
        np.broadcast_to((-0.5 * c2)[None, :], (CIN, CK))
    ).astype(np.float32)
    # -c2/2 split into bf16 hi + remainder lo so the f32r (tf32) matmul
    # fold is exact to ~2^-19; plus a ones row for the broadcast trick
    c2h = (-0.5 * c2).astype(np.float32)
    hi = c2h.astype(np.dtype("bfloat16") if hasattr(np, "bfloat16") else None)         if False else (
        np.frombuffer(
            (c2h.view(np.uint32) & np.uint32(0xFFFF0000)).tobytes(),
            dtype=np.float32,
        )
    )
    lo = c2h - hi
    c2row = np.zeros((1, 2 * CK + NT), dtype=np.float32)
    c2row[0, :CK] = hi
    c2row[0, CK : 2 * CK] = lo
    c2row[0, 2 * CK :] = 1.0

    # lut[c*K+k, o] = (centroids[c] @ weight[c])[k, o] + bias[o]/NC
    lut_full = np.einsum("cks,cso->cko", centroids, weight).reshape(CK, COUT)
    lut_full = lut_full + bias[None, :] / NC
    lut2 = np.concatenate([lut_full[:CIN], lut_full[CIN:]], axis=1)  # [128, 512]
    lut2 = np.ascontiguousarray(lut2).astype(np.float16)
    return cmm, c2g, lut2, c2row


def prep_x(xi):
    xp = np.zeros((CIN, XPL), dtype=np.float32)
    xp[:, 1 : 1 + PW * PW] = np.pad(xi, ((0, 0), (1, 1), (1, 1))).reshape(
        CIN, PW * PW
    )
    return xp


def prep_in_maps(x, centroids, weight, bias):
    x = np.asarray(x, dtype=np.float32)
    cmm, c2g, lut2, c2row = prep_consts(centroids, weight, bias)
    idn = np.eye(NT, dtype=np.float16)
    return [
        {
            "xp": prep_x(x[i]),
            "cmm": cmm,
            "c2g": c2g,
            "lut": lut2,
            "idn": idn,
            "c2row": c2row,
        }
        for i in range(B)
    ]


# valid-position selector over the 26*128 flat slots
_PFLAT = np.arange(P0, P0 + NTILES * NT)
_PSEL = (_PFLAT <= PLAST) & (_PFLAT % PW >= 1) & (_PFLAT % PW <= W)


def unpack_out(raw):
    """raw [CIN, NTILES*COUT] f16 -> [COUT, H, W] f32 for one image."""
    arr = np.asarray(raw, dtype=np.float32).reshape(CIN, NTILES, COUT)
    a = arr.transpose(1, 0, 2).reshape(NTILES * NT, COUT)  # flat slot-major
    return a[_PSEL].reshape(H, W, COUT).transpose(2, 0, 1)


_NC_CACHE = []


def kernel(x, centroids, weight, inverse_temperature_logit, bias):
    if not _NC_CACHE:
        _NC_CACHE.append(build())
    nc = _NC_CACHE[0]

    in_maps = prep_in_maps(x, centroids, weight, bias)
    res = run_bass_kernel_spmd(nc, in_maps, core_ids=list(range(B)))
    out = np.stack([unpack_out(res.results[i]["out"]) for i in range(B)])
    return np.ascontiguousarray(out.astype(np.float32))
